# revision 1
# baseline (speedup 1.0000x reference)
"""Trainium2 Bass kernel for AdaptiveChannelMixing.

Math (per batch b, query n):
  w    = (qc[n] @ W_gen + b_gen).reshape(G, GD, GD)        # [g, e, d]
  mixed[p, g*GD+e] = sum_d x[n, p, g*GD+d] * w[g, e, d]
  out  = relu(layer_norm(mixed) * gamma + beta)

Strategy: data-parallel over B (8 batches -> 8 NeuronCores). Host pre-lays-out
inputs (bf16 casts + transposes) so the device does zero transposes:
  - xT[chalf, dd, n*P+p]  : x with channels on partitions (dd = (g%2)*64+d)
  - qcT[c, n]             : qc transposed (moving operand of the gen matmul)
  - Wg[c, j]              : natural layout (stationary operand)
  - bT[dd, chalf*64+e]    : b_gen laid out to be a per-partition bias vector
Device per core:
  gen:   for each (chalf, e): psum[dd,n] = Wg_tile.T @ qcT  (K=256 in 2 steps),
         bias-add fused into the PSUM->SBUF copy (tensor_scalar / activation)
  mix:   per (n, g): psum[32p, 64e] += xT_blk.T @ wT_blk    (bf16)
  LN:    bn_stats/bn_aggr per 128-row tile, then one fused ACT op:
         relu((mixed - mu) * rs) via activation(Relu, scale=rs, bias=-mu*rs)
"""

import numpy as np
import ml_dtypes

B, N, P, C = 8, 300, 32, 256
G, GD = 4, 64
OUT_DIM = C * GD  # 16384
NP = N * P  # 9600
NCHUNKS = [152, 148]

_CACHE = {}


def _build_nc(trivial_affine: bool, reps: int = 1):
    from contextlib import ExitStack
    import concourse.bass as bass
    import concourse.tile as tile
    from concourse import bacc, mybir

    BF = mybir.dt.bfloat16
    F32 = mybir.dt.float32
    AF = mybir.ActivationFunctionType
    ALU = mybir.AluOpType

    nc = bacc.Bacc()
    xT = nc.declare_dram_parameter("xT", [2, 128, NP], BF, isOutput=False)
    qcT = nc.declare_dram_parameter("qcT", [C, N], BF, isOutput=False)
    Wg = nc.declare_dram_parameter("Wg", [C, OUT_DIM], BF, isOutput=False)
    bT = nc.declare_dram_parameter("bT", [128, 128], F32, isOutput=False)
    if not trivial_affine:
        gamma = nc.declare_dram_parameter("gamma", [C], F32, isOutput=False)
        beta = nc.declare_dram_parameter("beta", [C], F32, isOutput=False)
    out = nc.declare_dram_parameter("out", [NP, C], BF, isOutput=True)

    with tile.TileContext(nc) as tc, ExitStack() as ctx:
        singles = ctx.enter_context(tc.tile_pool(name="singles", bufs=1))
        xpool = ctx.enter_context(tc.tile_pool(name="xpool", bufs=2))
        wpool = ctx.enter_context(tc.tile_pool(name="wpool", bufs=2))
        gpsum = ctx.enter_context(tc.tile_pool(name="gpsum", bufs=3, space="PSUM"))
        mpsum = ctx.enter_context(tc.tile_pool(name="mpsum", bufs=2, space="PSUM"))
        spool = ctx.enter_context(tc.tile_pool(name="spool", bufs=8))
        opool = ctx.enter_context(tc.tile_pool(name="opool", bufs=4))

        # Resident weights: Wg as 2 K-tiles of [128, 16384] bf16.
        Wg_sb = []
        for ct in range(2):
            t = singles.tile([128, OUT_DIM], BF, tag=f"wg{ct}")
            # Chunked load so the first gen matmuls can start early.
            for jc in range(8):
                nc.sync.dma_start(
                    out=t[:, jc * 2048:(jc + 1) * 2048],
                    in_=Wg[ct * 128:(ct + 1) * 128, jc * 2048:(jc + 1) * 2048])
            # Host pre-permuted: col = chalf*8192 + e*128 + gsel*64 + d, so a
            # stationary tile is one contiguous 128-col run.
            Wg_sb.append(t.rearrange("p (h e m) -> p h e m", h=2, e=GD))
        qcT_sb = []
        for ct in range(2):
            t = singles.tile([128, N], BF, tag=f"qct{ct}")
            nc.sync.dma_start(out=t, in_=qcT[ct * 128:(ct + 1) * 128, :])
            qcT_sb.append(t)
        bT_sb = singles.tile([128, 128], F32, tag="bt")
        nc.sync.dma_start(out=bT_sb, in_=bT[:, :])
        eps_t = singles.tile([128, 1], F32, tag="eps")
        nc.vector.memset(eps_t, 1e-5)
        if not trivial_affine:
            gamma_ap = gamma[:]
            gamma_bc = singles.tile([128, C], F32, tag="gbc")
            nc.sync.dma_start(
                out=gamma_bc,
                in_=bass.AP(tensor=gamma_ap.tensor, offset=gamma_ap.offset,
                            ap=[[0, 128]] + list(gamma_ap.ap)),
            )
            beta_ap = beta[:]
            beta_bc = singles.tile([128, C], F32, tag="bbc")
            nc.sync.dma_start(
                out=beta_bc,
                in_=bass.AP(tensor=beta_ap.tensor, offset=beta_ap.offset,
                            ap=[[0, 128]] + list(beta_ap.ap)),
            )

        rep_ctx = tc.For_i(0, reps, 1) if reps > 1 else None
        if rep_ctx is not None:
            ctx.enter_context(rep_ctx)
        for _rep in range(1):
            npoff = 0
            noff = 0
            for ci, ncl in enumerate(NCHUNKS):
                npc = ncl * P
                xts = []
                wTs = []
                for chalf in range(2):
                    xt = xpool.tile([128, npc], BF, tag=f"xt{chalf}")
                    nc.sync.dma_start(out=xt, in_=xT[chalf, :, npoff:npoff + npc])
                    xts.append(xt)

                    wT = wpool.tile([128, GD, ncl], BF, tag=f"wT{chalf}")
                    for e in range(GD):
                        pw = gpsum.tile([128, ncl], F32, tag="pw")
                        for ct in range(2):
                            lhsT = Wg_sb[ct][:, chalf, e, :]
                            nc.tensor.matmul(
                                pw, lhsT=lhsT,
                                rhs=qcT_sb[ct][:, noff:noff + ncl],
                                start=(ct == 0), stop=(ct == 1),
                            )
                        bcol = bT_sb[:, chalf * GD + e:chalf * GD + e + 1]
                        if e % 2 == 0:
                            nc.vector.tensor_scalar_add(
                                out=wT[:, e, :], in0=pw, scalar1=bcol,
                            )
                        else:
                            nc.scalar.activation(
                                out=wT[:, e, :], in_=pw,
                                func=AF.Identity, bias=bcol, scale=1.0,
                            )
                    wTs.append(wT)

                for q in range(ncl // 4):
                    # pm is 2 PSUM banks; bank = g%2 so one bank only ever sees
                    # one PE row position (row-switch into the same bank crashes
                    # the device). col = (g%2)*512 + (g//2)*64 + e.
                    pm = mpsum.tile([128, 1024], F32, tag="pm")
                    for sub in range(4):
                        nl = q * 4 + sub
                        for chalf in range(2):
                            for g2 in range(2):
                                col = g2 * 512 + chalf * GD
                                lhsT = xts[chalf][g2 * GD:(g2 + 1) * GD,
                                                  nl * P:(nl + 1) * P]
                                rhs = wTs[chalf][g2 * GD:(g2 + 1) * GD, :, nl:nl + 1]
                                nc.tensor.matmul(
                                    pm[sub * P:(sub + 1) * P, col:col + GD],
                                    lhsT=lhsT, rhs=rhs, start=True, stop=True,
                                    tile_position=(g2 * GD, sub * P),
                                )
                    # c = g*64+e lives at pm col (g%2)*512 + (g//2)*64 + e:
                    # view as [128, b(2), hi(2), e(64)], c = b*64 + hi*128 + e.
                    pm_v = pm.rearrange("p (b r) -> p b r", b=2)[:, :, 0:128]
                    pm_r = pm_v.rearrange("p b (hi e) -> p b hi e", hi=2)
                    stats = spool.tile([128, 2, 6], F32, tag="stats")
                    for b2 in range(2):
                        nc.vector.bn_stats(out=stats[:, b2, :], in_=pm_v[:, b2, :])
                    mv = spool.tile([128, 2], F32, tag="mv")
                    nc.vector.bn_aggr(out=mv, in_=stats)
                    std = spool.tile([128, 1], F32, tag="std")
                    nc.scalar.activation(out=std, in_=mv[:, 1:2], func=AF.Sqrt,
                                         bias=eps_t, scale=1.0)
                    rs = spool.tile([128, 1], F32, tag="rs")
                    nc.vector.reciprocal(out=rs, in_=std)
                    nmr = spool.tile([128, 1], F32, tag="nmr")
                    nc.vector.tensor_scalar(
                        out=nmr, in0=mv[:, 0:1],
                        scalar1=rs, op0=ALU.mult,
                        scalar2=-1.0, op1=ALU.mult,
                    )
                    osb = opool.tile([128, C], BF, tag="osb")
                    # out iteration (b, hi, e) -> c = b*64 + hi*128 + e
                    osb_p = osb.rearrange("p (cb ch e) -> p ch cb e", cb=2, ch=2)
                    if trivial_affine:
                        nc.scalar.activation(out=osb_p, in_=pm_r, func=AF.Relu,
                                             bias=nmr, scale=rs)
                    else:
                        nc.scalar.activation(out=osb_p, in_=pm_r, func=AF.Identity,
                                             bias=nmr, scale=rs)
                        nc.vector.tensor_mul(osb, osb, gamma_bc)
                        nc.vector.tensor_add(osb, osb, beta_bc)
                        nc.vector.tensor_scalar_max(out=osb, in0=osb, scalar1=0.0)
                    row0 = npoff + q * 128
                    nc.sync.dma_start(out=out[row0:row0 + 128, :], in_=osb)

                npoff += npc
                noff += ncl

    nc.finalize()
    return nc


def _get_nc(trivial_affine: bool):
    key = ("nc", trivial_affine)
    if key not in _CACHE:
        _CACHE[key] = _build_nc(trivial_affine)
    return _CACHE[key]


def kernel(x, query_content, W_gen, b_gen, gamma, beta):
    from concourse.bass_utils import run_bass_kernel_spmd

    bf16 = ml_dtypes.bfloat16
    x = np.asarray(x, dtype=np.float32)
    qc = np.asarray(query_content, dtype=np.float32)
    W_gen = np.asarray(W_gen, dtype=np.float32)
    b_gen = np.asarray(b_gen, dtype=np.float32)
    gamma = np.asarray(gamma, dtype=np.float32)
    beta = np.asarray(beta, dtype=np.float32)

    trivial = bool(np.all(gamma == 1.0) and np.all(beta == 0.0))
    nc = _get_nc(trivial)

    # Shared (replicated) host-side layouts. W_gen columns permuted so that
    # each gen stationary tile is contiguous: col = ((chalf, e), (gsel, d)).
    Wg_perm = (W_gen.reshape(C, 2, 2, GD, GD)
               .transpose(0, 1, 3, 2, 4).reshape(C, OUT_DIM))
    Wg_bf = np.ascontiguousarray(Wg_perm.astype(bf16))
    b_r = b_gen.reshape(G, GD, GD)  # [g, e, d]
    bT = np.zeros((128, 128), dtype=np.float32)
    for g in range(G):
        bT[(g % 2) * GD:(g % 2 + 1) * GD,
           (g // 2) * GD:(g // 2 + 1) * GD] = b_r[g].T
    bT = np.ascontiguousarray(bT)

    in_maps = []
    for b in range(B):
        # xT[chalf, dd, n*P+p]: channels on partitions.
        xb = x[b].reshape(NP, 2, 128).transpose(1, 2, 0)
        im = {
            "xT": np.ascontiguousarray(xb.astype(bf16)),
            "qcT": np.ascontiguousarray(qc[b].T.astype(bf16)),
            "Wg": Wg_bf,
            "bT": bT,
        }
        if not trivial:
            im["gamma"] = gamma
            im["beta"] = beta
        in_maps.append(im)

    import os
    trace = bool(os.environ.get("BASS_TRACE"))
    res = run_bass_kernel_spmd(nc, in_maps, core_ids=list(range(B)), trace=trace)
    _CACHE["last_res"] = res
    out = np.stack([res.results[i]["out"].reshape(N, P, C) for i in range(B)])
    return out.astype(np.float32)

